# revision 13
# baseline (speedup 1.0000x reference)
"""Trainium2 Bass kernel for CustomMultiHeadSelfAttention.

Problem shapes: B=4, N=2048, E=1024, H=16, HD=64.

Sharding (8 cores): core c -> batch b = c//2, head-group g = c%2
(heads 8g..8g+7, i.e. feature cols [512g, 512g+512) of q/k/v).

v2 schedule: the ACT exp stream (256 tiles x ~1.1us) is the pacing
engine, so the kernel is organized to keep it busy from ~15us on:
  - attention unit (q0, p0) starts as soon as the first xt quarter and
    pair-0 QK weights land; V / K projections for later key blocks are
    emitted just-in-time inside that unit's key-block loop.
  - all remaining projection work (Q/K tiles for pairs 1-3, the
    out-projection) is chopped into <=2-matmul jobs dripped into the
    per-keyblock slack; a force-drain before each unit guarantees
    emission-order correctness (PE queue is FIFO).
  - V bias is folded into the host-side output bias (softmax weights
    sum to 1, so attn(V + b) = attn(V) + b), removing the device adds.
  - a quarter of the exp tiles (kb % 4 == 2, units >= 1) are computed
    on the idle DVE via a bf16 Schraudolph bit-trick exp
    (i16 = round(s * 0.125 * 128/ln2 + (127*128 - C)); bitcast bf16),
    relieving the ACT bottleneck; softmax renormalization keeps the
    added relative error ~1e-2 vs the 2e-2 budget.
  - final q-chunk out-projection: pairs 0-2 contracted during the last
    unit, only the pair-3 term + per-tb flushes remain in the tail.

All matmuls run in bf16 with fp32 PSUM accumulation; softmax
statistics stay fp32.  PV matmuls use a 128-col stationary
[V_h | ones] so every PV also produces that head's softmax row-sums
(stationary width is free: matmul cost = moving free dim).
"""

import sys

if "/opt/trn_rl_repo" not in sys.path:
    sys.path.insert(0, "/opt/trn_rl_repo")

from collections import deque
from contextlib import ExitStack

import ml_dtypes
import numpy as np

import concourse.tile as tile
from concourse import bacc, mybir
from concourse.bass_utils import run_bass_kernel_spmd

B, N, E, H = 4, 2048, 1024, 16
HD = E // H          # 64
HL = H // 2          # 8 local heads per core
EL = HL * HD         # 512 local feature cols per core
NP = 128             # partitions
NPAIRS = HL // 2     # 4 head pairs per core
QC = 512             # query chunk (free dim of S^T / PV matmuls)
NQC = N // QC        # 4
NKB = N // NP        # 16 key blocks of 128
TC = 512             # token chunk in projections
EC = E // NP         # 8 contraction chunks in the in-projection

BF16 = mybir.dt.bfloat16
FP32 = mybir.dt.float32
I16 = mybir.dt.int16

# Schraudolph bf16 exp: exp(0.125*s) ~= bitcast_bf16(int16(s*A + B))
SCH_A = 0.125 * 128.0 / float(np.log(2.0))   # 23.0831...
SCH_C = 7.5                                   # rms-optimal rounding offset
SCH_B = 127.0 * 128.0 - SCH_C
# which key blocks use the DVE exp (units >= 1): kb % 4 == 2
SCH_KB = frozenset((2, 6, 10, 14))

_CACHED = {}


def build_kernel():
    nc = bacc.Bacc("TRN2", target_bir_lowering=False, debug=False, num_devices=8)

    xt_d = nc.dram_tensor("xt", [NP, EC * N], BF16, kind="ExternalInput").ap()
    wt_d = nc.dram_tensor("wt", [NP, EC * 3 * EL], BF16, kind="ExternalInput").ap()
    wot_d = nc.dram_tensor("wot", [NP, NPAIRS * E], BF16, kind="ExternalInput").ap()
    bias_d = nc.dram_tensor(
        "bias", [NP, 2 * NPAIRS + EL], FP32, kind="ExternalInput"
    ).ap()
    y_d = nc.dram_tensor("y", [NP, NKB, E], FP32, kind="ExternalOutput").ap()

    with tile.TileContext(nc) as tc:
        _emit(tc, xt_d, wt_d, wot_d, bias_d, y_d)
    nc.compile()
    return nc


def _emit(tc, xt_d, wt_d, wot_d, bias_d, y_d):
    nc = tc.nc
    ctx = ExitStack()
    with ctx:
        singles = ctx.enter_context(tc.tile_pool(name="singles", bufs=1))
        proj_ps = ctx.enter_context(tc.tile_pool(name="proj_ps", bufs=2, space="PSUM"))
        s_ps = ctx.enter_context(tc.tile_pool(name="s_ps", bufs=2, space="PSUM"))
        pv_ps = ctx.enter_context(tc.tile_pool(name="pv_ps", bufs=1, space="PSUM"))
        pv2_ps = ctx.enter_context(tc.tile_pool(name="pv2_ps", bufs=1, space="PSUM"))
        epool = ctx.enter_context(tc.tile_pool(name="epool", bufs=8))
        rpool = ctx.enter_context(tc.tile_pool(name="rpool", bufs=2))
        ypool = ctx.enter_context(tc.tile_pool(name="ypool", bufs=1))

        # ---- resident SBUF tensors -----------------------------------------
        xt4_sb = singles.tile([NP, 4, EC, N // 4], BF16)
        wtv_sb = singles.tile([NP, EC, EL], BF16)
        wt_sb = singles.tile([NP, EC, 2 * EL], BF16)   # pair-major [q_p|k_p]*4
        wot_sb = singles.tile([NP, NPAIRS, E], BF16)
        qt_sb = singles.tile([NP, NPAIRS, N], BF16)
        kt_sb = singles.tile([NP, NPAIRS, N], BF16)
        vo_sb = singles.tile([NP, NKB, HL, NP], BF16)
        at_sb = singles.tile([NP, NPAIRS, N], BF16)
        bias_sb = singles.tile([NP, 2 * NPAIRS + EL], FP32)

        xt_dv = xt_d.rearrange("p (tq ec t) -> p tq ec t", tq=4, ec=EC)
        wtqk_dv = wt_d[:, EC * EL:].rearrange("p (ec c) -> p ec c", ec=EC)

        # DMAs in need order: first scores need wtv+xt q0+pair-0 weights.
        nc.sync.dma_start(bias_sb[:], bias_d)
        nc.sync.dma_start(
            wtv_sb[:],
            wt_d[:, 0:EC * EL].rearrange("p (ec c) -> p ec c", ec=EC))
        nc.sync.dma_start(xt4_sb[:, 0], xt_dv[:, 0])
        nc.sync.dma_start(wt_sb[:, :, 0:2 * NP], wtqk_dv[:, :, 0:2 * NP])
        for tq in range(1, 4):
            nc.sync.dma_start(xt4_sb[:, tq], xt_dv[:, tq])
        nc.sync.dma_start(wt_sb[:, :, 2 * NP:], wtqk_dv[:, :, 2 * NP:])
        nc.sync.dma_start(wot_sb[:], wot_d.rearrange("p (pr j) -> p pr j", pr=NPAIRS))
        bqk_sb = bias_sb[:, 0:2 * NPAIRS]

        # ones for the fused row-sum columns; V data halves overwritten later
        nc.vector.memset(vo_sb[:], 1.0)

        # ---- projection job pieces -----------------------------------------
        def v_half(tb, half, box):
            """V projection for token block tb, contraction half."""
            if half == 0:
                ps = proj_ps.tile([NP, EL], FP32, tag="ps")
                box["ps"] = ps
            else:
                ps = box["ps"]
            for ec in range(4 * half, 4 * half + 4):
                nc.tensor.matmul(
                    ps[:],
                    lhsT=xt4_sb[:, tb // 4, ec, (tb % 4) * NP:(tb % 4 + 1) * NP],
                    rhs=wtv_sb[:, ec, :],
                    start=(ec == 0), stop=(ec == EC - 1),
                )
            if half == 1:
                psv = ps[:].rearrange("p (h two d) -> p h two d", two=2, d=HD)
                vov = vo_sb[:, tb].rearrange("p (h two) f -> p h two f", two=2)
                nc.vector.tensor_copy(vov[:, :, 0, 0:HD], psv[:, :, 0, :])
                nc.vector.tensor_copy(vov[:, :, 1, HD:NP], psv[:, :, 1, :])

        def v_job(tb):
            box = {}
            v_half(tb, 0, box)
            v_half(tb, 1, box)

        def qk_quarter(p, which, t, stage, box):
            """2-matmul stage of a Q/K tile (p, t); stage 3 adds bias."""
            # pair-major weights: Q(p) at col 2p*NP, K(p) at (2p+1)*NP
            coff = (2 * p + which) * NP
            dst = qt_sb if which == 0 else kt_sb
            bcol = p if which == 0 else NPAIRS + p
            if stage == 0:
                ps = proj_ps.tile([NP, TC], FP32, tag="ps")
                box["ps"] = ps
            else:
                ps = box["ps"]
            for ec in range(2 * stage, 2 * stage + 2):
                nc.tensor.matmul(
                    ps[:],
                    lhsT=wt_sb[:, ec, coff:coff + NP],
                    rhs=xt4_sb[:, t, ec, :],
                    start=(ec == 0), stop=(ec == EC - 1),
                )
            if stage == 3:
                nc.vector.tensor_tensor(
                    dst[:, p, t * TC:(t + 1) * TC], ps[:],
                    bqk_sb[:, bcol:bcol + 1].to_broadcast((NP, TC)),
                    mybir.AluOpType.add,
                )

        def qk_tile_jobs(p, which, t):
            box = {}
            for stage in range(4):
                yield (430 if stage < 3 else 1120,
                       lambda p=p, w=which, t=t, s=stage, b=box:
                       qk_quarter(p, w, t, s, b))

        def qk_tile_inline(p, which, t):
            box = {}
            for stage in range(4):
                qk_quarter(p, which, t, stage, box)

        # ---- out-projection jobs -------------------------------------------
        yb_state = {}

        def op_half(q, i, tb, jc, half, box):
            if half == 0:
                ps = proj_ps.tile([NP, TC], FP32, tag="ps")
                box["ps"] = ps
            else:
                ps = box["ps"]
            for p in (0, 1) if half == 0 else (2, 3):
                nc.tensor.matmul(
                    ps[:],
                    lhsT=at_sb[:, p, tb * NP:(tb + 1) * NP],
                    rhs=wot_sb[:, p, jc * TC:(jc + 1) * TC],
                    start=(p == 0), stop=(p == NPAIRS - 1),
                )
            if half == 1:
                yb, done = yb_state[q]
                nc.vector.tensor_copy(yb[:, i, jc * TC:(jc + 1) * TC], ps[:])
                done[0] += 1
                if done[0] == 4:
                    nc.sync.dma_start(
                        y_d[:, q * NQC:q * NQC + 2, :], yb[:, 0:2, :])
                elif done[0] == 8:
                    nc.sync.dma_start(
                        y_d[:, q * NQC + 2:q * NQC + 4, :], yb[:, 2:4, :])

        def outproj_jobs(q):
            yb = ypool.tile([NP, NQC, E], FP32, tag="yb")
            yb_state[q] = (yb, [0])
            for i, tb in enumerate(range(q * NQC, (q + 1) * NQC)):
                for jc in range(E // TC):
                    box = {}
                    for half in range(2):
                        yield (430 if half == 0 else 1120,
                               lambda q=q, i=i, tb=tb, jc=jc, h=half, b=box:
                               op_half(q, i, tb, jc, h, b))

        # final q-chunk: pairs 0-2 during the last unit, pair 3 in the tail
        ybF_box = {}

        def op3_partial(i, tb, jc, half, box):
            ybF = ybF_box["t"]
            if half == 0:
                ps = proj_ps.tile([NP, TC], FP32, tag="ps")
                box["ps"] = ps
                for p in (0, 1):
                    nc.tensor.matmul(
                        ps[:],
                        lhsT=at_sb[:, p, tb * NP:(tb + 1) * NP],
                        rhs=wot_sb[:, p, jc * TC:(jc + 1) * TC],
                        start=(p == 0), stop=False,
                    )
            else:
                ps = box["ps"]
                nc.tensor.matmul(
                    ps[:],
                    lhsT=at_sb[:, 2, tb * NP:(tb + 1) * NP],
                    rhs=wot_sb[:, 2, jc * TC:(jc + 1) * TC],
                    start=False, stop=True,
                )
                nc.vector.tensor_copy(ybF[:, i, jc * TC:(jc + 1) * TC], ps[:])

        def op3_partial_jobs():
            ybF = ypool.tile([NP, NQC, E], FP32, tag="yb")
            ybF_box["t"] = ybF
            q = NQC - 1
            for i, tb in enumerate(range(q * NQC, (q + 1) * NQC)):
                for jc in range(E // TC):
                    box = {}
                    for half in range(2):
                        yield (430 if half == 0 else 900,
                               lambda i=i, tb=tb, jc=jc, h=half, b=box:
                               op3_partial(i, tb, jc, h, b))

        def op3_final():
            ybF = ybF_box["t"]
            q = NQC - 1
            for i, tb in enumerate(range(q * NQC, (q + 1) * NQC)):
                for jc in range(E // TC):
                    ps = proj_ps.tile([NP, TC], FP32, tag="ps")
                    nc.tensor.matmul(
                        ps[:],
                        lhsT=at_sb[:, NPAIRS - 1, tb * NP:(tb + 1) * NP],
                        rhs=wot_sb[:, NPAIRS - 1, jc * TC:(jc + 1) * TC],
                        start=True, stop=True,
                    )
                    ybs = ybF[:, i, jc * TC:(jc + 1) * TC]
                    nc.vector.tensor_tensor(ybs, ybs, ps[:], mybir.AluOpType.add)
                nc.sync.dma_start(
                    y_d[:, q * NQC + i:q * NQC + i + 1, :], ybF[:, i:i + 1, :])

        # ---- job queue -------------------------------------------------------
        jobq = deque()           # items: (cost_ns, fn, tag_done_or_None)
        tags_done = set()

        def enqueue(gen, tag=None):
            items = list(gen)
            for k, (cost, fn) in enumerate(items):
                jobq.append((cost, fn, tag if k == len(items) - 1 else None))

        def run_job():
            cost, fn, tag = jobq.popleft()
            fn()
            if tag is not None:
                tags_done.add(tag)
            return cost

        def drain_until(tag):
            if tag in tags_done:
                return
            while jobq:
                _, _, t = jobq[0]
                run_job()
                if t == tag:
                    return

        def pop_budget(budget):
            spent = 0
            while jobq and spent < budget:
                if spent and spent + jobq[0][0] > 1600:
                    break
                spent += run_job()

        # ---- one attention unit ---------------------------------------------
        def attn_unit(q, p, ui, jit_unit0=False):
            qs = slice(q * QC, (q + 1) * QC)
            pvA = pv_ps.tile([NP, QC], FP32, tag="pv")
            pvB = pv2_ps.tile([NP, QC], FP32, tag="pv2")
            for g2 in range(NKB // 2):
                if jit_unit0:
                    # JIT V/K for upcoming key blocks (first unit only)
                    kb0 = 2 * g2
                    for kb in (kb0, kb0 + 1):
                        if 2 <= kb + 1 <= 15:
                            v_job(kb + 1)
                        if kb in (2, 6, 10):        # K(p0, t) one chunk ahead
                            qk_tile_inline(0, 1, kb // 4 + 1)
                    if g2 >= 6:
                        pop_budget(900)
                st0 = s_ps.tile([NP, 2, QC], FP32, tag="st")
                st1 = s_ps.tile([NP, 2, QC], FP32, tag="st")
                et0 = epool.tile([NP, 2, QC], BF16, tag="et")
                et1 = epool.tile([NP, 2, QC], BF16, tag="et")
                for j, st in ((0, st0), (1, st1)):
                    kb = 2 * g2 + j
                    ks = slice(kb * NP, (kb + 1) * NP)
                    nc.tensor.matmul(
                        st[:, 0, :],
                        lhsT=kt_sb[0:HD, p, ks], rhs=qt_sb[0:HD, p, qs],
                        start=True, stop=True,
                    )
                    nc.tensor.matmul(
                        st[:, 1, :],
                        lhsT=kt_sb[HD:NP, p, ks], rhs=qt_sb[HD:NP, p, qs],
                        start=True, stop=True,
                    )
                for j, st, et in ((0, st0, et0), (1, st1, et1)):
                    kb = 2 * g2 + j
                    if ui >= 1 and kb in SCH_KB:
                        # Schraudolph bf16 exp on the DVE
                        nc.vector.tensor_scalar(
                            et[:].bitcast(I16), st[:],
                            SCH_A, SCH_B,
                            mybir.AluOpType.mult, mybir.AluOpType.add,
                        )
                    else:
                        nc.scalar.activation(
                            et[:], st[:], mybir.ActivationFunctionType.Exp,
                            scale=0.125,
                        )
                for j, et in ((0, et0), (1, et1)):
                    kb = 2 * g2 + j
                    first, last = (kb == 0), (kb == NKB - 1)
                    nc.tensor.matmul(
                        pvA[:],
                        lhsT=vo_sb[:, kb, 2 * p, :],
                        rhs=et[:, 0, :], start=first, stop=last,
                    )
                    nc.tensor.matmul(
                        pvB[:],
                        lhsT=vo_sb[:, kb, 2 * p + 1, :],
                        rhs=et[:, 1, :], start=first, stop=last,
                    )
                if not jit_unit0:
                    pop_budget(1100)
            # normalize: evacuate PV banks, reciprocals, scale into at_sb
            cA = rpool.tile([NP, QC], FP32, tag="cA")
            cB = rpool.tile([NP, QC], FP32, tag="cB")
            nc.vector.tensor_copy(cA[:], pvA[:])
            nc.vector.tensor_copy(cB[:], pvB[:])
            rcA = rpool.tile([NP, QC], FP32, tag="rcA")
            rcB = rpool.tile([NP, QC], FP32, tag="rcB")
            rc2 = rpool.tile([NP, QC], FP32, tag="rc2")
            nc.vector.reciprocal_approx_fast(rcA[:], cA[:])
            nc.vector.reciprocal_approx_fast(rcB[:], cB[:])
            nc.sync.dma_start(rc2[0:HD, :], rcA[HD:NP, :])
            nc.sync.dma_start(rc2[HD:NP, :], rcB[0:HD, :])
            nc.vector.tensor_mul(at_sb[0:HD, p, qs], cA[0:HD, :], rc2[0:HD, :])
            nc.vector.tensor_mul(at_sb[HD:NP, p, qs], cB[HD:NP, :],
                                 rc2[HD:NP, :])

        # ---- schedule --------------------------------------------------------
        # (q, p) unit order: pairs introduced progressively so K/Q drip fits;
        # q0/q2/q1 complete early (idx 9/10/11) so their out-projections can
        # drip across the remaining units; q3 uses the partial-pairs tail.
        sched = [(0, 0), (1, 0), (2, 0), (0, 1), (2, 1), (1, 1), (0, 2),
                 (2, 2), (1, 2), (0, 3), (2, 3), (1, 3), (3, 0), (3, 1),
                 (3, 2), (3, 3)]

        # prologue: V for the first key blocks, pair-0 K/Q for chunk 0
        v_job(0)
        v_job(1)
        qk_tile_inline(0, 1, 0)       # K(p0, t0)
        qk_tile_inline(0, 0, 0)       # Q(p0, q0)

        # drip enqueue plan: unit idx -> list of (gen, tag) to enqueue at start
        def qk_gen(p, which, t):
            return qk_tile_jobs(p, which, t)

        # tag Q{p}{q} = Q tile of pair p, query chunk q; unit (q,p) needs it
        enq_plan = {
            0: [(qk_gen(0, 0, 1), "Q01")],
            1: [(qk_gen(0, 0, 2), "Q02")]
               + [(qk_gen(1, 1, t), "K1" if t == 3 else None) for t in range(4)]
               + [(qk_gen(1, 0, 0), "Q10")],
            2: [(qk_gen(1, 0, 2), "Q12"), (qk_gen(1, 0, 1), "Q11")],
            4: [(qk_gen(2, 1, t), "K2" if t == 3 else None) for t in range(4)]
               + [(qk_gen(2, 0, 0), "Q20"), (qk_gen(2, 0, 2), "Q22")],
            6: [(qk_gen(2, 0, 1), "Q21")]
               + [(qk_gen(3, 1, t), "K3" if t == 3 else None) for t in range(4)]
               + [(qk_gen(3, 0, 0), "Q30")],
            8: [(qk_gen(3, 0, 2), "Q32"), (qk_gen(3, 0, 1), "Q31")],
            10: [(outproj_jobs(0), "OP0"), (qk_gen(0, 0, 3), "Q03"),
                 (qk_gen(1, 0, 3), "Q13")],
            11: [(outproj_jobs(2), "OP2")],
            12: [(outproj_jobs(1), "OP1"), (qk_gen(2, 0, 3), "Q23"),
                 (qk_gen(3, 0, 3), "Q33")],
            15: [(op3_partial_jobs(), "OP3P")],
        }
        prereq = {
            (0, 0): [], (1, 0): ["Q01"], (2, 0): ["Q02"],
            (0, 1): ["K1", "Q10"], (2, 1): ["Q12"], (1, 1): ["Q11"],
            (0, 2): ["K2", "Q20"], (2, 2): ["Q22"], (1, 2): ["Q21"],
            (0, 3): ["K3", "Q30"], (2, 3): ["Q32"], (1, 3): ["Q31"],
            (3, 0): ["Q03"], (3, 1): ["Q13"], (3, 2): ["Q23"],
            (3, 3): ["Q33"],
        }

        for ui, (q, p) in enumerate(sched):
            for gen, tag in enq_plan.get(ui, []):
                enqueue(gen, tag)
            for tag in prereq[(q, p)]:
                drain_until(tag)
            attn_unit(q, p, ui, jit_unit0=(ui == 0))

        # drain any leftover jobs (op3 partials etc.), then the tail
        while jobq:
            run_job()
        op3_final()


def shard_inputs(qkv, in_proj_w, in_proj_b, out_proj_w):
    """Build the 8 per-core input maps (host-side transpose + bf16 cast)."""
    bf = ml_dtypes.bfloat16
    in_maps = []
    for c in range(8):
        b, g = c // 2, c % 2
        cs = slice(g * EL, (g + 1) * EL)
        xt = np.ascontiguousarray(
            qkv[b].T.reshape(EC, NP, 4, N // 4).transpose(1, 2, 0, 3)
            .reshape(NP, EC * N)
        ).astype(bf)
        wq_l = in_proj_w[cs]                    # [EL, E]
        wk_l = in_proj_w[E:2 * E][cs]           # [EL, E]
        wv_l = in_proj_w[2 * E:3 * E][cs]       # [EL, E]
        # pair-major qk section: [q_p0 | k_p0 | q_p1 | k_p1 | ...]
        qk_rows = []
        for p in range(NPAIRS):
            qk_rows.append(wq_l[p * NP:(p + 1) * NP])
            qk_rows.append(wk_l[p * NP:(p + 1) * NP])
        w_l = np.concatenate(qk_rows, 0)        # [2*EL, E]
        wtv = wv_l.T.reshape(EC, NP, EL).transpose(1, 0, 2).reshape(NP, -1)
        wtqk = w_l.T.reshape(EC, NP, 2 * EL).transpose(1, 0, 2).reshape(NP, -1)
        wt = np.ascontiguousarray(
            np.concatenate([wtv, wtqk], axis=1)
        ).astype(bf)
        wot = np.ascontiguousarray(
            out_proj_w[:, cs].T.reshape(NPAIRS, NP, E).transpose(1, 0, 2)
            .reshape(NP, -1)
        ).astype(bf)
        bias = np.zeros((NP, 2 * NPAIRS + EL), np.float32)
        bq = in_proj_b[cs]
        bk = in_proj_b[E:2 * E][cs]
        for p in range(NPAIRS):
            bias[:, p] = bq[p * NP:(p + 1) * NP]
            bias[:, NPAIRS + p] = bk[p * NP:(p + 1) * NP]
        in_maps.append({"xt": xt, "wt": wt, "wot": wot, "bias": bias})
    return in_maps


def unshard_output(ys, in_proj_b, out_proj_w, out_proj_b):
    # V bias folded out of the device kernel: attn(V+b) = attn(V) + b,
    # so y gets + b_v @ W_out^T once per batch (cores' halves sum to full).
    bv_term = in_proj_b[2 * E:3 * E] @ out_proj_w.T
    full = [np.asarray(y).transpose(1, 0, 2).reshape(N, E) for y in ys]
    out = np.stack([full[2 * b] + full[2 * b + 1] for b in range(B)])
    out += (out_proj_b + bv_term)[None, None, :]
    return out.astype(np.float32)


def kernel(qkv, in_proj_w, in_proj_b, out_proj_w, out_proj_b):
    qkv = np.asarray(qkv, np.float32)
    in_proj_w = np.asarray(in_proj_w, np.float32)
    in_proj_b = np.asarray(in_proj_b, np.float32)
    out_proj_w = np.asarray(out_proj_w, np.float32)
    out_proj_b = np.asarray(out_proj_b, np.float32)

    if "nc" not in _CACHED:
        _CACHED["nc"] = build_kernel()
    nc = _CACHED["nc"]

    in_maps = shard_inputs(qkv, in_proj_w, in_proj_b, out_proj_w)
    res = run_bass_kernel_spmd(nc, in_maps, core_ids=list(range(8)))
    ys = [res.results[c]["y"] for c in range(8)]
    return unshard_output(ys, in_proj_b, out_proj_w, out_proj_b)


# revision 17
# speedup vs baseline: 1.0235x; 1.0235x over previous
"""Trainium2 Bass kernel for CustomMultiHeadSelfAttention.

Problem shapes: B=4, N=2048, E=1024, H=16, HD=64.

Sharding (8 cores): core c -> batch b = c//2, head-group g = c%2
(heads 8g..8g+7, i.e. feature cols [512g, 512g+512) of q/k/v).

v2 schedule: the ACT exp stream (256 tiles x ~1.1us) is the pacing
engine, so the kernel is organized to keep it busy from ~15us on:
  - attention unit (q0, p0) starts as soon as the first xt quarter and
    pair-0 QK weights land; V / K projections for later key blocks are
    emitted just-in-time inside that unit's key-block loop.
  - all remaining projection work (Q/K tiles for pairs 1-3, the
    out-projection) is chopped into <=2-matmul jobs dripped into the
    per-keyblock slack; a force-drain before each unit guarantees
    emission-order correctness (PE queue is FIFO).
  - V bias is folded into the host-side output bias (softmax weights
    sum to 1, so attn(V + b) = attn(V) + b), removing the device adds.
  - a quarter of the exp tiles (kb % 4 == 2, units >= 1) are computed
    on the idle DVE via a bf16 Schraudolph bit-trick exp
    (i16 = round(s * 0.125 * 128/ln2 + (127*128 - C)); bitcast bf16),
    relieving the ACT bottleneck; softmax renormalization keeps the
    added relative error ~1e-2 vs the 2e-2 budget.
  - final q-chunk out-projection: pairs 0-2 contracted during the last
    unit, only the pair-3 term + per-tb flushes remain in the tail.

All matmuls run in bf16 with fp32 PSUM accumulation; softmax
statistics stay fp32.  PV matmuls use a 128-col stationary
[V_h | ones] so every PV also produces that head's softmax row-sums
(stationary width is free: matmul cost = moving free dim).
"""

import sys

if "/opt/trn_rl_repo" not in sys.path:
    sys.path.insert(0, "/opt/trn_rl_repo")

from collections import deque
from contextlib import ExitStack

import ml_dtypes
import numpy as np

import concourse.tile as tile
from concourse import bacc, mybir
from concourse.bass_utils import run_bass_kernel_spmd

B, N, E, H = 4, 2048, 1024, 16
HD = E // H          # 64
HL = H // 2          # 8 local heads per core
EL = HL * HD         # 512 local feature cols per core
NP = 128             # partitions
NPAIRS = HL // 2     # 4 head pairs per core
QC = 512             # query chunk (free dim of S^T / PV matmuls)
NQC = N // QC        # 4
NKB = N // NP        # 16 key blocks of 128
TC = 512             # token chunk in projections
EC = E // NP         # 8 contraction chunks in the in-projection

BF16 = mybir.dt.bfloat16
FP32 = mybir.dt.float32
I16 = mybir.dt.int16

# Schraudolph bf16 exp: exp(0.125*s) ~= bitcast_bf16(int16(s*A + B))
SCH_A = 0.125 * 128.0 / float(np.log(2.0))   # 23.0831...
SCH_C = 7.5                                   # rms-optimal rounding offset
SCH_B = 127.0 * 128.0 - SCH_C
# which key blocks use the DVE exp (units >= 1)
SCH_KB = frozenset((6, 14))

_CACHED = {}


def build_kernel():
    nc = bacc.Bacc("TRN2", target_bir_lowering=False, debug=False, num_devices=8)

    xt_d = nc.dram_tensor("xt", [NP, EC * N], BF16, kind="ExternalInput").ap()
    wt_d = nc.dram_tensor("wt", [NP, EC * 3 * EL], BF16, kind="ExternalInput").ap()
    wot_d = nc.dram_tensor("wot", [NP, NPAIRS * E], BF16, kind="ExternalInput").ap()
    bias_d = nc.dram_tensor(
        "bias", [NP, 2 * NPAIRS + EL], FP32, kind="ExternalInput"
    ).ap()
    y_d = nc.dram_tensor("y", [NP, NKB, E], FP32, kind="ExternalOutput").ap()

    with tile.TileContext(nc) as tc:
        _emit(tc, xt_d, wt_d, wot_d, bias_d, y_d)
    nc.compile()
    return nc


def _emit(tc, xt_d, wt_d, wot_d, bias_d, y_d):
    nc = tc.nc
    ctx = ExitStack()
    with ctx:
        singles = ctx.enter_context(tc.tile_pool(name="singles", bufs=1))
        proj_ps = ctx.enter_context(tc.tile_pool(name="proj_ps", bufs=2, space="PSUM"))
        s_ps = ctx.enter_context(tc.tile_pool(name="s_ps", bufs=2, space="PSUM"))
        pv_ps = ctx.enter_context(tc.tile_pool(name="pv_ps", bufs=1, space="PSUM"))
        pv2_ps = ctx.enter_context(tc.tile_pool(name="pv2_ps", bufs=1, space="PSUM"))
        epool = ctx.enter_context(tc.tile_pool(name="epool", bufs=8))
        rpool = ctx.enter_context(tc.tile_pool(name="rpool", bufs=2))
        ypool = ctx.enter_context(tc.tile_pool(name="ypool", bufs=1))

        # ---- resident SBUF tensors -----------------------------------------
        xt4_sb = singles.tile([NP, 4, EC, N // 4], BF16)
        wtv_sb = singles.tile([NP, EC, EL], BF16)
        wt_sb = singles.tile([NP, EC, 2 * EL], BF16)   # pair-major [q_p|k_p]*4
        wot_sb = singles.tile([NP, NPAIRS, E], BF16)
        qt_sb = singles.tile([NP, NPAIRS, N], BF16)
        kt_sb = singles.tile([NP, NPAIRS, N], BF16)
        vo_sb = singles.tile([NP, NKB, HL, NP], BF16)
        at_sb = singles.tile([NP, NPAIRS, N], BF16)
        bias_sb = singles.tile([NP, 2 * NPAIRS + EL], FP32)

        xt_dv = xt_d.rearrange("p (tq ec t) -> p tq ec t", tq=4, ec=EC)
        wtqk_dv = wt_d[:, EC * EL:].rearrange("p (ec c) -> p ec c", ec=EC)

        # DMAs in need order: first scores need wtv+xt q0+pair-0 weights.
        nc.sync.dma_start(bias_sb[:], bias_d)
        nc.sync.dma_start(
            wtv_sb[:],
            wt_d[:, 0:EC * EL].rearrange("p (ec c) -> p ec c", ec=EC))
        nc.sync.dma_start(xt4_sb[:, 0], xt_dv[:, 0])
        nc.sync.dma_start(wt_sb[:, :, 0:2 * NP], wtqk_dv[:, :, 0:2 * NP])
        for tq in range(1, 4):
            nc.sync.dma_start(xt4_sb[:, tq], xt_dv[:, tq])
        nc.sync.dma_start(wt_sb[:, :, 2 * NP:], wtqk_dv[:, :, 2 * NP:])
        nc.sync.dma_start(wot_sb[:], wot_d.rearrange("p (pr j) -> p pr j", pr=NPAIRS))
        bqk_sb = bias_sb[:, 0:2 * NPAIRS]

        # ones for the fused row-sum columns; V data halves overwritten later
        nc.vector.memset(vo_sb[:], 1.0)

        # ---- projection job pieces -----------------------------------------
        def v_half(tb, half, box):
            """V projection for token block tb, contraction half."""
            if half == 0:
                ps = proj_ps.tile([NP, EL], FP32, tag="ps")
                box["ps"] = ps
            else:
                ps = box["ps"]
            for ec in range(4 * half, 4 * half + 4):
                nc.tensor.matmul(
                    ps[:],
                    lhsT=xt4_sb[:, tb // 4, ec, (tb % 4) * NP:(tb % 4 + 1) * NP],
                    rhs=wtv_sb[:, ec, :],
                    start=(ec == 0), stop=(ec == EC - 1),
                )
            if half == 1:
                psv = ps[:].rearrange("p (h two d) -> p h two d", two=2, d=HD)
                vov = vo_sb[:, tb].rearrange("p (h two) f -> p h two f", two=2)
                nc.vector.tensor_copy(vov[:, :, 0, 0:HD], psv[:, :, 0, :])
                nc.vector.tensor_copy(vov[:, :, 1, HD:NP], psv[:, :, 1, :])

        def v_job(tb):
            box = {}
            v_half(tb, 0, box)
            v_half(tb, 1, box)

        def qk_quarter(p, which, t, stage, box):
            """2-matmul stage of a Q/K tile (p, t); stage 3 adds bias."""
            # pair-major weights: Q(p) at col 2p*NP, K(p) at (2p+1)*NP
            coff = (2 * p + which) * NP
            dst = qt_sb if which == 0 else kt_sb
            bcol = p if which == 0 else NPAIRS + p
            if stage == 0:
                ps = proj_ps.tile([NP, TC], FP32, tag="ps")
                box["ps"] = ps
            else:
                ps = box["ps"]
            for ec in range(2 * stage, 2 * stage + 2):
                nc.tensor.matmul(
                    ps[:],
                    lhsT=wt_sb[:, ec, coff:coff + NP],
                    rhs=xt4_sb[:, t, ec, :],
                    start=(ec == 0), stop=(ec == EC - 1),
                )
            if stage == 3:
                nc.vector.tensor_tensor(
                    dst[:, p, t * TC:(t + 1) * TC], ps[:],
                    bqk_sb[:, bcol:bcol + 1].to_broadcast((NP, TC)),
                    mybir.AluOpType.add,
                )

        def qk_tile_jobs(p, which, t):
            box = {}
            for stage in range(4):
                yield (430 if stage < 3 else 1120,
                       lambda p=p, w=which, t=t, s=stage, b=box:
                       qk_quarter(p, w, t, s, b))

        def qk_tile_inline(p, which, t):
            box = {}
            for stage in range(4):
                qk_quarter(p, which, t, stage, box)

        # ---- out-projection jobs -------------------------------------------
        yb_state = {}

        def op_half(q, i, tb, jc, half, box):
            if half == 0:
                ps = proj_ps.tile([NP, TC], FP32, tag="ps")
                box["ps"] = ps
            else:
                ps = box["ps"]
            for p in (0, 1) if half == 0 else (2, 3):
                nc.tensor.matmul(
                    ps[:],
                    lhsT=at_sb[:, p, tb * NP:(tb + 1) * NP],
                    rhs=wot_sb[:, p, jc * TC:(jc + 1) * TC],
                    start=(p == 0), stop=(p == NPAIRS - 1),
                )
            if half == 1:
                yb, done = yb_state[q]
                nc.vector.tensor_copy(yb[:, i, jc * TC:(jc + 1) * TC], ps[:])
                done[0] += 1
                if done[0] == 4:
                    nc.sync.dma_start(
                        y_d[:, q * NQC:q * NQC + 2, :], yb[:, 0:2, :])
                elif done[0] == 8:
                    nc.sync.dma_start(
                        y_d[:, q * NQC + 2:q * NQC + 4, :], yb[:, 2:4, :])

        def outproj_jobs(q):
            yb = ypool.tile([NP, NQC, E], FP32, tag="yb")
            yb_state[q] = (yb, [0])
            for i, tb in enumerate(range(q * NQC, (q + 1) * NQC)):
                for jc in range(E // TC):
                    box = {}
                    for half in range(2):
                        yield (430 if half == 0 else 1120,
                               lambda q=q, i=i, tb=tb, jc=jc, h=half, b=box:
                               op_half(q, i, tb, jc, h, b))

        # final q-chunk: pairs 0-2 during the last unit, pair 3 in the tail
        ybF_box = {}

        def op3_partial(i, tb, jc, half, box):
            ybF = ybF_box["t"]
            if half == 0:
                ps = proj_ps.tile([NP, TC], FP32, tag="ps")
                box["ps"] = ps
                for p in (0, 1):
                    nc.tensor.matmul(
                        ps[:],
                        lhsT=at_sb[:, p, tb * NP:(tb + 1) * NP],
                        rhs=wot_sb[:, p, jc * TC:(jc + 1) * TC],
                        start=(p == 0), stop=False,
                    )
            else:
                ps = box["ps"]
                nc.tensor.matmul(
                    ps[:],
                    lhsT=at_sb[:, 2, tb * NP:(tb + 1) * NP],
                    rhs=wot_sb[:, 2, jc * TC:(jc + 1) * TC],
                    start=False, stop=True,
                )
                nc.vector.tensor_copy(ybF[:, i, jc * TC:(jc + 1) * TC], ps[:])

        def op3_partial_jobs():
            ybF = ypool.tile([NP, NQC, E], FP32, tag="yb")
            ybF_box["t"] = ybF
            q = NQC - 1
            for i, tb in enumerate(range(q * NQC, (q + 1) * NQC)):
                for jc in range(E // TC):
                    box = {}
                    for half in range(2):
                        yield (430 if half == 0 else 900,
                               lambda i=i, tb=tb, jc=jc, h=half, b=box:
                               op3_partial(i, tb, jc, h, b))

        def op3_final():
            ybF = ybF_box["t"]
            q = NQC - 1
            for i, tb in enumerate(range(q * NQC, (q + 1) * NQC)):
                for jc in range(E // TC):
                    ps = proj_ps.tile([NP, TC], FP32, tag="ps")
                    nc.tensor.matmul(
                        ps[:],
                        lhsT=at_sb[:, NPAIRS - 1, tb * NP:(tb + 1) * NP],
                        rhs=wot_sb[:, NPAIRS - 1, jc * TC:(jc + 1) * TC],
                        start=True, stop=True,
                    )
                    ybs = ybF[:, i, jc * TC:(jc + 1) * TC]
                    nc.vector.tensor_tensor(ybs, ybs, ps[:], mybir.AluOpType.add)
                nc.sync.dma_start(
                    y_d[:, q * NQC + i:q * NQC + i + 1, :], ybF[:, i:i + 1, :])

        # ---- job queue -------------------------------------------------------
        jobq = deque()           # items: (cost_ns, fn, tag_done_or_None)
        tags_done = set()

        def enqueue(gen, tag=None):
            items = list(gen)
            for k, (cost, fn) in enumerate(items):
                jobq.append((cost, fn, tag if k == len(items) - 1 else None))

        def run_job():
            cost, fn, tag = jobq.popleft()
            fn()
            if tag is not None:
                tags_done.add(tag)
            return cost

        def drain_until(tag):
            if tag in tags_done:
                return
            while jobq:
                _, _, t = jobq[0]
                run_job()
                if t == tag:
                    return

        def pop_budget(budget):
            spent = 0
            while jobq and spent < budget:
                if spent and spent + jobq[0][0] > 1600:
                    break
                spent += run_job()

        # ---- one attention unit ---------------------------------------------
        def attn_unit(q, p, ui, jit_unit0=False):
            qs = slice(q * QC, (q + 1) * QC)
            pvA = pv_ps.tile([NP, QC], FP32, tag="pv")
            pvB = pv2_ps.tile([NP, QC], FP32, tag="pv2")
            for g2 in range(NKB // 2):
                if jit_unit0:
                    # JIT V/K for upcoming key blocks (first unit only)
                    kb0 = 2 * g2
                    for kb in (kb0, kb0 + 1):
                        if 2 <= kb + 1 <= 15:
                            v_job(kb + 1)
                        if kb in (2, 6, 10):        # K(p0, t) one chunk ahead
                            qk_tile_inline(0, 1, kb // 4 + 1)
                    if g2 >= 6:
                        pop_budget(900)
                st0 = s_ps.tile([NP, 2, QC], FP32, tag="st")
                st1 = s_ps.tile([NP, 2, QC], FP32, tag="st")
                et0 = epool.tile([NP, 2, QC], BF16, tag="et")
                et1 = epool.tile([NP, 2, QC], BF16, tag="et")
                for j, st, et in ((0, st0, et0), (1, st1, et1)):
                    kb = 2 * g2 + j
                    ks = slice(kb * NP, (kb + 1) * NP)
                    nc.tensor.matmul(
                        st[:, 0, :],
                        lhsT=kt_sb[0:HD, p, ks], rhs=qt_sb[0:HD, p, qs],
                        start=True, stop=True,
                    )
                    nc.tensor.matmul(
                        st[:, 1, :],
                        lhsT=kt_sb[HD:NP, p, ks], rhs=qt_sb[HD:NP, p, qs],
                        start=True, stop=True,
                    )
                    if ui >= 1 and kb in SCH_KB:
                        # Schraudolph bf16 exp on the DVE
                        nc.vector.tensor_scalar(
                            et[:].bitcast(I16), st[:],
                            SCH_A, SCH_B,
                            mybir.AluOpType.mult, mybir.AluOpType.add,
                        )
                    else:
                        nc.scalar.activation(
                            et[:], st[:], mybir.ActivationFunctionType.Exp,
                            scale=0.125,
                        )
                # drip jobs here: they fill the PE while exp results arrive
                if not jit_unit0:
                    pop_budget(1100)
                for j, et in ((0, et0), (1, et1)):
                    kb = 2 * g2 + j
                    first, last = (kb == 0), (kb == NKB - 1)
                    nc.tensor.matmul(
                        pvA[:],
                        lhsT=vo_sb[:, kb, 2 * p, :],
                        rhs=et[:, 0, :], start=first, stop=last,
                    )
                    nc.tensor.matmul(
                        pvB[:],
                        lhsT=vo_sb[:, kb, 2 * p + 1, :],
                        rhs=et[:, 1, :], start=first, stop=last,
                    )
            # normalize: evacuate PV banks, reciprocals, scale into at_sb
            cA = rpool.tile([NP, QC], FP32, tag="cA")
            cB = rpool.tile([NP, QC], FP32, tag="cB")
            nc.vector.tensor_copy(cA[:], pvA[:])
            nc.vector.tensor_copy(cB[:], pvB[:])
            rcA = rpool.tile([NP, QC], FP32, tag="rcA")
            rcB = rpool.tile([NP, QC], FP32, tag="rcB")
            rc2 = rpool.tile([NP, QC], FP32, tag="rc2")
            nc.vector.reciprocal_approx_fast(rcA[:], cA[:])
            nc.vector.reciprocal_approx_fast(rcB[:], cB[:])
            nc.sync.dma_start(rc2[0:HD, :], rcA[HD:NP, :])
            nc.sync.dma_start(rc2[HD:NP, :], rcB[0:HD, :])
            nc.vector.tensor_mul(at_sb[0:HD, p, qs], cA[0:HD, :], rc2[0:HD, :])
            nc.vector.tensor_mul(at_sb[HD:NP, p, qs], cB[HD:NP, :],
                                 rc2[HD:NP, :])

        # ---- schedule --------------------------------------------------------
        # (q, p) unit order: pairs introduced progressively so K/Q drip fits;
        # q0/q2/q1 complete early (idx 9/10/11) so their out-projections can
        # drip across the remaining units; q3 uses the partial-pairs tail.
        sched = [(0, 0), (1, 0), (2, 0), (0, 1), (2, 1), (1, 1), (0, 2),
                 (2, 2), (1, 2), (0, 3), (2, 3), (1, 3), (3, 0), (3, 1),
                 (3, 2), (3, 3)]

        # prologue: V for the first key blocks, pair-0 K/Q for chunk 0
        v_job(0)
        v_job(1)
        qk_tile_inline(0, 1, 0)       # K(p0, t0)
        qk_tile_inline(0, 0, 0)       # Q(p0, q0)

        # drip enqueue plan: unit idx -> list of (gen, tag) to enqueue at start
        def qk_gen(p, which, t):
            return qk_tile_jobs(p, which, t)

        # tag Q{p}{q} = Q tile of pair p, query chunk q; unit (q,p) needs it
        enq_plan = {
            0: [(qk_gen(0, 0, 1), "Q01")],
            1: [(qk_gen(0, 0, 2), "Q02")]
               + [(qk_gen(1, 1, t), "K1" if t == 3 else None) for t in range(4)]
               + [(qk_gen(1, 0, 0), "Q10")],
            2: [(qk_gen(1, 0, 2), "Q12"), (qk_gen(1, 0, 1), "Q11")],
            4: [(qk_gen(2, 1, t), "K2" if t == 3 else None) for t in range(4)]
               + [(qk_gen(2, 0, 0), "Q20"), (qk_gen(2, 0, 2), "Q22")],
            6: [(qk_gen(2, 0, 1), "Q21")]
               + [(qk_gen(3, 1, t), "K3" if t == 3 else None) for t in range(4)]
               + [(qk_gen(3, 0, 0), "Q30")],
            8: [(qk_gen(3, 0, 2), "Q32"), (qk_gen(3, 0, 1), "Q31")],
            10: [(qk_gen(0, 0, 3), "Q03"), (qk_gen(1, 0, 3), "Q13"),
                 (outproj_jobs(0), "OP0")],
            11: [(outproj_jobs(2), "OP2")],
            12: [(qk_gen(2, 0, 3), "Q23"), (qk_gen(3, 0, 3), "Q33"),
                 (outproj_jobs(1), "OP1")],
            15: [(op3_partial_jobs(), "OP3P")],
        }
        prereq = {
            (0, 0): [], (1, 0): ["Q01"], (2, 0): ["Q02"],
            (0, 1): ["K1", "Q10"], (2, 1): ["Q12"], (1, 1): ["Q11"],
            (0, 2): ["K2", "Q20"], (2, 2): ["Q22"], (1, 2): ["Q21"],
            (0, 3): ["K3", "Q30"], (2, 3): ["Q32"], (1, 3): ["Q31"],
            (3, 0): ["Q03"], (3, 1): ["Q13"], (3, 2): ["Q23"],
            (3, 3): ["Q33"],
        }

        for ui, (q, p) in enumerate(sched):
            for gen, tag in enq_plan.get(ui, []):
                enqueue(gen, tag)
            for tag in prereq[(q, p)]:
                drain_until(tag)
            attn_unit(q, p, ui, jit_unit0=(ui == 0))

        # drain any leftover jobs (op3 partials etc.), then the tail
        while jobq:
            run_job()
        op3_final()


def shard_inputs(qkv, in_proj_w, in_proj_b, out_proj_w):
    """Build the 8 per-core input maps (host-side transpose + bf16 cast)."""
    bf = ml_dtypes.bfloat16
    in_maps = []
    for c in range(8):
        b, g = c // 2, c % 2
        cs = slice(g * EL, (g + 1) * EL)
        xt = np.ascontiguousarray(
            qkv[b].T.reshape(EC, NP, 4, N // 4).transpose(1, 2, 0, 3)
            .reshape(NP, EC * N)
        ).astype(bf)
        wq_l = in_proj_w[cs]                    # [EL, E]
        wk_l = in_proj_w[E:2 * E][cs]           # [EL, E]
        wv_l = in_proj_w[2 * E:3 * E][cs]       # [EL, E]
        # pair-major qk section: [q_p0 | k_p0 | q_p1 | k_p1 | ...]
        qk_rows = []
        for p in range(NPAIRS):
            qk_rows.append(wq_l[p * NP:(p + 1) * NP])
            qk_rows.append(wk_l[p * NP:(p + 1) * NP])
        w_l = np.concatenate(qk_rows, 0)        # [2*EL, E]
        wtv = wv_l.T.reshape(EC, NP, EL).transpose(1, 0, 2).reshape(NP, -1)
        wtqk = w_l.T.reshape(EC, NP, 2 * EL).transpose(1, 0, 2).reshape(NP, -1)
        wt = np.ascontiguousarray(
            np.concatenate([wtv, wtqk], axis=1)
        ).astype(bf)
        wot = np.ascontiguousarray(
            out_proj_w[:, cs].T.reshape(NPAIRS, NP, E).transpose(1, 0, 2)
            .reshape(NP, -1)
        ).astype(bf)
        bias = np.zeros((NP, 2 * NPAIRS + EL), np.float32)
        bq = in_proj_b[cs]
        bk = in_proj_b[E:2 * E][cs]
        for p in range(NPAIRS):
            bias[:, p] = bq[p * NP:(p + 1) * NP]
            bias[:, NPAIRS + p] = bk[p * NP:(p + 1) * NP]
        in_maps.append({"xt": xt, "wt": wt, "wot": wot, "bias": bias})
    return in_maps


def unshard_output(ys, in_proj_b, out_proj_w, out_proj_b):
    # V bias folded out of the device kernel: attn(V+b) = attn(V) + b,
    # so y gets + b_v @ W_out^T once per batch (cores' halves sum to full).
    bv_term = in_proj_b[2 * E:3 * E] @ out_proj_w.T
    full = [np.asarray(y).transpose(1, 0, 2).reshape(N, E) for y in ys]
    out = np.stack([full[2 * b] + full[2 * b + 1] for b in range(B)])
    out += (out_proj_b + bv_term)[None, None, :]
    return out.astype(np.float32)


def kernel(qkv, in_proj_w, in_proj_b, out_proj_w, out_proj_b):
    qkv = np.asarray(qkv, np.float32)
    in_proj_w = np.asarray(in_proj_w, np.float32)
    in_proj_b = np.asarray(in_proj_b, np.float32)
    out_proj_w = np.asarray(out_proj_w, np.float32)
    out_proj_b = np.asarray(out_proj_b, np.float32)

    if "nc" not in _CACHED:
        _CACHED["nc"] = build_kernel()
    nc = _CACHED["nc"]

    in_maps = shard_inputs(qkv, in_proj_w, in_proj_b, out_proj_w)
    res = run_bass_kernel_spmd(nc, in_maps, core_ids=list(range(8)))
    ys = [res.results[c]["y"] for c in range(8)]
    return unshard_output(ys, in_proj_b, out_proj_w, out_proj_b)
